# revision 5
# baseline (speedup 1.0000x reference)
"""Trainium2 Bass kernel for nn_CMFA (dense_transformer, seq_len=1 cross-attention).

Math notes (exact simplifications vs the reference):
  - softmax over a single key is exactly 1.0, so the attention output is
    exactly the v-projection: mha(q,k,v) = (v @ Wv.T + bv) @ Wo.T + bo.
    The q/k projections never influence the output.
  - Wv -> Wo -> fi2 is a linear chain (no nonlinearity), so it is folded on
    the host:  V = [v1, i_] @ Wcat.T + bcat  with
      Wcat = [fi2 @ (Wo @ Wv), fi2],  bcat = fi2 @ (Wo @ bv + bo) + fi2_b
    (the i_ column block carries the residual through fi2).

Device layout: activations are feature-major ("transposed", [feat, batch]) so
every matmul contracts over the partition dim and every DMA is contiguous.
The host pre-transposes the batch shards of i/t and transposes the output
back. Pure data parallel across 8 cores; weights replicated.

DMAs are batched 4 K-chunks (1MB) per transfer into [128, 4, NT] tiles,
relying on Tile's subtile dependency tracking so each matmul only waits on
its own slice. Input loads for batch-tile n+1 are emitted right after tile
n's fi1 matmuls so the in-order Sync dispatch queue prefetches them ahead
of tile n's output stores.
"""

import numpy as np

B, IMG, TAB, HID = 32768, 2048, 128, 512
NCORES = 8
BS = B // NCORES  # rows per core
NT = 512          # batch-tile (matmul moving/free dim)

_CACHE = {}


def _pack_blocks(WT: np.ndarray, K: int, M: int) -> np.ndarray:
    """[K*128, M*128] -> [128, K*M*128] with col ((k*M+m)*128 + j) = WT[k*128+p, m*128+j]."""
    out = WT.reshape(K, 128, M, 128).transpose(1, 0, 2, 3).reshape(128, K * M * 128)
    return np.ascontiguousarray(out, dtype=np.float32)


def _build_nc(bs: int):
    import concourse.bass as bass
    import concourse.tile as tile
    from concourse import bacc, mybir

    f32 = mybir.dt.float32
    f32r = mybir.dt.float32r
    Relu = mybir.ActivationFunctionType.Relu
    Ident = mybir.ActivationFunctionType.Identity
    ntiles = bs // NT

    nc = bacc.Bacc("TRN2", target_bir_lowering=False, debug=False)

    iT_d = nc.dram_tensor("iT", [IMG, bs], f32r, kind="ExternalInput").ap()
    tT_d = nc.dram_tensor("tT", [TAB, bs], f32r, kind="ExternalInput").ap()
    w_fi1_d = nc.dram_tensor("w_fi1", [128, 64 * 128], f32r, kind="ExternalInput").ap()
    w_ft1_d = nc.dram_tensor("w_ft1", [128, 4 * 128], f32r, kind="ExternalInput").ap()
    w_ci1_d = nc.dram_tensor("w_ci1", [128, 16 * 128], f32r, kind="ExternalInput").ap()
    w_ct1_d = nc.dram_tensor("w_ct1", [128, 16 * 128], f32r, kind="ExternalInput").ap()
    w_V_d = nc.dram_tensor("w_V", [128, 32 * 128], f32r, kind="ExternalInput").ap()
    w_T_d = nc.dram_tensor("w_T", [128, 32 * 128], f32r, kind="ExternalInput").ap()
    bias_d = nc.dram_tensor("bias", [128, 24], f32, kind="ExternalInput").ap()
    out_d = nc.dram_tensor("outT", [2 * HID, bs], f32, kind="ExternalOutput").ap()

    # grouped views: row (512g + 128j + p) <-> [g, p, j]
    iT_r = iT_d.rearrange("(g j p) b -> g p j b", j=4, p=128)    # [4,128,4,bs]
    out_r = out_d.rearrange("(s j p) b -> s p j b", j=4, p=128)  # [2,128,4,bs]

    with tile.TileContext(nc) as tc:
        with (
            tc.tile_pool(name="w", bufs=1) as wpool,
            tc.tile_pool(name="x", bufs=6) as xpool,
            tc.tile_pool(name="h", bufs=1) as hpool,
            tc.tile_pool(name="o", bufs=2) as opool,
            tc.tile_pool(name="ps", bufs=2, space="PSUM") as pspool,
        ):
            # weight tiles: grouped 4 k-chunks = [128, 2048] per tile
            wf1 = [wpool.tile([128, 2048], f32r, name=f"w_fi1_{g}") for g in range(4)]
            wt1 = wpool.tile([128, 512], f32r, name="w_ft1_0")
            wc1 = wpool.tile([128, 2048], f32r, name="w_ci1_0")
            wc2 = wpool.tile([128, 2048], f32r, name="w_ct1_0")
            wV = [wpool.tile([128, 2048], f32r, name=f"w_V_{g}") for g in range(2)]
            wT = [wpool.tile([128, 2048], f32r, name=f"w_T_{g}") for g in range(2)]
            bt = wpool.tile([128, 24], f32, name="bias_t")

            def xload(n):
                """4 DMAs of [128, 4, NT] covering iT k-chunks for batch-tile n."""
                c0 = n * NT
                xs = []
                for g in range(4):
                    xb = xpool.tile([128, 4, NT], f32r, tag="x", name=f"x_{n}_{g}")
                    nc.sync.dma_start(xb[:], iT_r[g, :, :, c0:c0 + NT])
                    xs.append(xb)
                return xs

            # preamble: interleave first x groups with fi1 weight groups
            nc.sync.dma_start(bt[:], bias_d[:])
            x_cur = []
            for g in range(4):
                xb = xpool.tile([128, 4, NT], f32r, tag="x", name=f"x_0_{g}")
                nc.sync.dma_start(xb[:], iT_r[g, :, :, 0:NT])
                nc.sync.dma_start(wf1[g][:], w_fi1_d[:, 2048 * g:2048 * (g + 1)])
                x_cur.append(xb)
            xt_cur = xpool.tile([128, NT], f32r, tag="xt", bufs=2, name="xt_0")
            nc.sync.dma_start(xt_cur[:], tT_d[:, 0:NT])
            nc.sync.dma_start(wt1[:], w_ft1_d[:])
            nc.sync.dma_start(wc1[:], w_ci1_d[:])
            nc.sync.dma_start(wc2[:], w_ct1_d[:])
            for g in range(2):
                nc.sync.dma_start(wV[g][:], w_V_d[:, 2048 * g:2048 * (g + 1)])
                nc.sync.dma_start(wT[g][:], w_T_d[:, 2048 * g:2048 * (g + 1)])

            def wslice(wtile, kk, m):
                # k-chunk kk (within tile), m-block m -> [128, 128]
                off = kk * 512 + m * 128
                return wtile[:, off:off + 128]

            for n in range(ntiles):
                c0 = n * NT
                # ---- i_ = relu(i @ fi1.T + b) ----
                ps1 = pspool.tile([128, 4, NT], f32, tag="ps", name=f"ps1_{n}")
                for k in range(16):
                    xs = x_cur[k // 4][:, k % 4, :]
                    for m in range(4):
                        nc.tensor.matmul(ps1[:, m, :], wslice(wf1[k // 4], k % 4, m),
                                         xs, start=(k == 0), stop=(k == 15))

                # prefetch next tile's inputs (early in Sync program order)
                if n + 1 < ntiles:
                    x_nxt = xload(n + 1)
                    xt_nxt = xpool.tile([128, NT], f32r, tag="xt", bufs=2,
                                        name=f"xt_{n + 1}")
                    nc.sync.dma_start(xt_nxt[:], tT_d[:, c0 + NT:c0 + 2 * NT])

                i_ = hpool.tile([128, 4, NT], f32r, tag="i_", name=f"i__{n}")
                for m in range(4):
                    nc.scalar.activation(i_[:, m, :], ps1[:, m, :], Relu,
                                         bias=bt[:, m:m + 1])

                # ---- t_ = relu(t @ ft1.T + b) ----
                ps2 = pspool.tile([128, 4, NT], f32, tag="ps", name=f"ps2_{n}")
                for m in range(4):
                    nc.tensor.matmul(ps2[:, m, :], wslice(wt1, 0, m), xt_cur[:],
                                     start=True, stop=True)
                t_ = hpool.tile([128, 4, NT], f32r, tag="t_", name=f"t__{n}")
                for m in range(4):
                    nc.scalar.activation(t_[:, m, :], ps2[:, m, :], Relu,
                                         bias=bt[:, 4 + m:5 + m])

                # ---- v1 = relu(i_ @ ci1.T + b) ----
                ps3 = pspool.tile([128, 4, NT], f32, tag="ps", name=f"ps3_{n}")
                for k in range(4):
                    for m in range(4):
                        nc.tensor.matmul(ps3[:, m, :], wslice(wc1, k, m),
                                         i_[:, k, :], start=(k == 0), stop=(k == 3))
                v1 = hpool.tile([128, 4, NT], f32r, tag="v1", name=f"v1_{n}")
                for m in range(4):
                    nc.scalar.activation(v1[:, m, :], ps3[:, m, :], Relu,
                                         bias=bt[:, 8 + m:9 + m])

                # ---- v2 = relu(t_ @ ct1.T + b) ----
                ps4 = pspool.tile([128, 4, NT], f32, tag="ps", name=f"ps4_{n}")
                for k in range(4):
                    for m in range(4):
                        nc.tensor.matmul(ps4[:, m, :], wslice(wc2, k, m),
                                         t_[:, k, :], start=(k == 0), stop=(k == 3))
                v2 = hpool.tile([128, 4, NT], f32r, tag="v2", name=f"v2_{n}")
                for m in range(4):
                    nc.scalar.activation(v2[:, m, :], ps4[:, m, :], Relu,
                                         bias=bt[:, 12 + m:13 + m])

                # ---- V = [v1, i_] @ WcatV.T + bcatV ----
                psV = pspool.tile([128, 4, NT], f32, tag="ps", name=f"psV_{n}")
                for k in range(4):
                    for m in range(4):
                        nc.tensor.matmul(psV[:, m, :], wslice(wV[0], k, m),
                                         v1[:, k, :], start=(k == 0), stop=False)
                for k in range(4):
                    for m in range(4):
                        nc.tensor.matmul(psV[:, m, :], wslice(wV[1], k, m),
                                         i_[:, k, :], start=False, stop=(k == 3))
                oV = opool.tile([128, 4, NT], f32, tag="o", name=f"oV_{n}")
                for m in range(4):
                    nc.scalar.activation(oV[:, m, :], psV[:, m, :], Ident,
                                         bias=bt[:, 16 + m:17 + m])
                nc.sync.dma_start(out_r[0, :, :, c0:c0 + NT], oV[:])

                # ---- T = [v2, t_] @ WcatT.T + bcatT ----
                psT = pspool.tile([128, 4, NT], f32, tag="ps", name=f"psT_{n}")
                for k in range(4):
                    for m in range(4):
                        nc.tensor.matmul(psT[:, m, :], wslice(wT[0], k, m),
                                         v2[:, k, :], start=(k == 0), stop=False)
                for k in range(4):
                    for m in range(4):
                        nc.tensor.matmul(psT[:, m, :], wslice(wT[1], k, m),
                                         t_[:, k, :], start=False, stop=(k == 3))
                oT = opool.tile([128, 4, NT], f32, tag="o", name=f"oT_{n}")
                for m in range(4):
                    nc.scalar.activation(oT[:, m, :], psT[:, m, :], Ident,
                                         bias=bt[:, 20 + m:21 + m])
                nc.sync.dma_start(out_r[1, :, :, c0:c0 + NT], oT[:])

                if n + 1 < ntiles:
                    x_cur = x_nxt
                    xt_cur = xt_nxt

    nc.compile()
    return nc


def _host_pack(inp: dict):
    f8 = np.float64
    fi1_w, fi1_b = inp["fi1_w"], inp["fi1_b"]
    ft1_w, ft1_b = inp["ft1_w"], inp["ft1_b"]
    ci1_w, ci1_b = inp["ci1_w"], inp["ci1_b"]
    ct1_w, ct1_b = inp["ct1_w"], inp["ct1_b"]

    def fold(wv, bv, wo, bo, f_w, f_b):
        Wvo = wo.astype(f8) @ wv.astype(f8)
        bvo = wo.astype(f8) @ bv.astype(f8) + bo.astype(f8)
        Wcat = np.concatenate([f_w.astype(f8) @ Wvo, f_w.astype(f8)], axis=1)
        bcat = f_w.astype(f8) @ bvo + f_b.astype(f8)
        return Wcat.astype(np.float32), bcat.astype(np.float32)

    WcatV, bcatV = fold(inp["aV_wv"], inp["aV_bv"], inp["aV_wo"], inp["aV_bo"],
                        inp["fi2_w"], inp["fi2_b"])
    WcatT, bcatT = fold(inp["aT_wv"], inp["aT_bv"], inp["aT_wo"], inp["aT_bo"],
                        inp["ft2_w"], inp["ft2_b"])

    weights = {
        "w_fi1": _pack_blocks(np.ascontiguousarray(fi1_w.T), 16, 4),
        "w_ft1": _pack_blocks(np.ascontiguousarray(ft1_w.T), 1, 4),
        "w_ci1": _pack_blocks(np.ascontiguousarray(ci1_w.T), 4, 4),
        "w_ct1": _pack_blocks(np.ascontiguousarray(ct1_w.T), 4, 4),
        "w_V": _pack_blocks(np.ascontiguousarray(WcatV.T), 8, 4),
        "w_T": _pack_blocks(np.ascontiguousarray(WcatT.T), 8, 4),
    }
    cols = []
    for b in (fi1_b, ft1_b, ci1_b, ct1_b, bcatV, bcatT):
        for m in range(4):
            cols.append(b[128 * m:128 * (m + 1)])
    weights["bias"] = np.ascontiguousarray(np.stack(cols, axis=1), dtype=np.float32)
    return weights


def kernel(**inputs) -> np.ndarray:
    from concourse import bass_utils

    i = np.asarray(inputs["i"], dtype=np.float32)
    t = np.asarray(inputs["t"], dtype=np.float32)
    weights = _host_pack(inputs)

    if "nc" not in _CACHE:
        _CACHE["nc"] = _build_nc(BS)
    nc = _CACHE["nc"]

    in_maps = []
    for c in range(NCORES):
        sl = slice(c * BS, (c + 1) * BS)
        m = dict(weights)
        m["iT"] = np.ascontiguousarray(i[sl].T)
        m["tT"] = np.ascontiguousarray(t[sl].T)
        in_maps.append(m)

    res = bass_utils.run_bass_kernel_spmd(nc, in_maps, core_ids=list(range(NCORES)))

    out = np.empty((B, 2 * HID), dtype=np.float32)
    for c in range(NCORES):
        out[c * BS:(c + 1) * BS] = res.results[c]["outT"].T
    return out
